# revision 28
# baseline (speedup 1.0000x reference)
"""Trainium2 Bass kernel for DFlashAttention (self-contained).

Sharding: 8 cores = 2 batches x 4 head-groups (tensor-parallel over heads).
Each core handles 8 q-heads / 2 kv-heads of one batch, computes a partial
output through Wo (input-dim sharded); host sums the 4 partials per batch.

Projections run as float32r matmuls (full PE speed, ~tf32 precision) with
activations/weights transposed on host. RMS-norm + RoPE run in the transposed
layout via PE tricks (ones-column sum matmuls, broadcast matmuls,
permutation-matrix rotate-half with signs and gamma folded into precomputed
cos/sin). The attention part (QK^T scores -> exp -> AV -> Wo) runs in bf16
operands with fp32 PSUM accumulation; softmax over t (the partition dim)
skips max-subtraction (scores bounded ~+-6) and gets its denominator free
from a ones-row appended to V, applied after AV via a broadcast matmul +
fast reciprocal. Emission is stage-pipelined so PE never stalls on ACT/DVE.
"""

import numpy as np
import ml_dtypes

NUM_HEADS = 32
NUM_KV_HEADS = 8
HEAD_DIM = 64
EPS = 1e-6
B, S, CTX, HID = 2, 1024, 1024, 2048
T = CTX + S  # 2048

_CACHE = {}


def _build():
    import concourse.bass as bass
    import concourse.mybir as mybir
    import concourse.tile as tile
    from concourse import bacc

    f32 = mybir.dt.float32
    f32r = mybir.dt.float32r
    bf16 = mybir.dt.bfloat16
    Act = mybir.ActivationFunctionType

    nc = bacc.Bacc("TRN2", target_bir_lowering=False, debug=False)

    # ---- DRAM I/O ----
    xt_d = nc.dram_tensor("xt", [HID, T], f32, kind="ExternalInput")
    wqt_d = nc.dram_tensor("wqt", [HID, 512], f32, kind="ExternalInput")
    wkt_d = nc.dram_tensor("wkt", [HID, 128], f32, kind="ExternalInput")
    wvt_d = nc.dram_tensor("wvt", [HID, 128], f32, kind="ExternalInput")
    wot_d = nc.dram_tensor("wot", [512, HID], bf16, kind="ExternalInput")
    cosq_d = nc.dram_tensor("cosq", [128, S], f32, kind="ExternalInput")
    sinq_d = nc.dram_tensor("sinq", [128, S], f32, kind="ExternalInput")
    cosk_d = nc.dram_tensor("cosk", [128, T], f32, kind="ExternalInput")
    sink_d = nc.dram_tensor("sink", [128, T], f32, kind="ExternalInput")
    prot_d = nc.dram_tensor("prot", [128, 128], f32, kind="ExternalInput")
    onesblk_d = nc.dram_tensor("onesblk", [128, 2], f32, kind="ExternalInput")
    ident_d = nc.dram_tensor("ident", [128, 128], f32, kind="ExternalInput")
    bones_d = nc.dram_tensor("bones", [1, 64], f32, kind="ExternalInput")
    o_d = nc.dram_tensor("o", [S, HID], f32, kind="ExternalOutput")
    DEBUG = _CACHE.get("debug", False)
    if DEBUG:
        qf_dbg = nc.dram_tensor("qf_dbg", [4, 128, S], bf16, kind="ExternalOutput")
        kf_dbg = nc.dram_tensor("kf_dbg", [128, T], bf16, kind="ExternalOutput")
        vpr_dbg = nc.dram_tensor("vpr_dbg", [2, 128, 16, 65], bf16, kind="ExternalOutput")
        attn_dbg = nc.dram_tensor("attn_dbg", [4, 128, S], bf16, kind="ExternalOutput")
        nb_dbg = {}
        for nm, shp, dt_ in [("qcp0", [128, 512], bf16), ("sq0", [128, 512], bf16),
                             ("s00", [1, 512], f32), ("sc00", [1, 512], f32),
                             ("b00", [64, 512], f32), ("bc0", [128, 512], f32),
                             ("qn0", [128, 512], f32), ("rot0", [128, 512], f32),
                             ("m10", [128, 512], f32), ("m20", [128, 512], f32)]:
            nb_dbg[nm] = nc.dram_tensor(nm, shp, dt_, kind="ExternalOutput")

    with tile.TileContext(nc) as tc:
        with (
            tc.tile_pool(name="consts", bufs=1) as cpool,
            tc.tile_pool(name="long", bufs=1) as lpool,
            tc.tile_pool(name="work", bufs=1) as wk_pool,
        ):
            # ---- constants ----
            prot = cpool.tile([128, 128], f32r)
            nc.sync.dma_start(prot[:], prot_d[:].bitcast(f32r))
            onesblk = cpool.tile([128, 2], f32r)
            nc.sync.dma_start(onesblk[:], onesblk_d[:].bitcast(f32r))
            ident = cpool.tile([128, 128], f32)
            nc.sync.dma_start(ident[:], ident_d[:])
            bones = cpool.tile([1, 64], f32r)
            nc.sync.dma_start(bones[:], bones_d[:].bitcast(f32r))
            eps_t = cpool.tile([1, 1], f32)
            nc.vector.memset(eps_t[:], EPS)
            ones_col = cpool.tile([128, 16, 1], f32)
            nc.vector.memset(ones_col[:], 1.0)
            cosq = cpool.tile([128, S], f32)
            sinq = cpool.tile([128, S], f32)
            cosk = cpool.tile([128, T], f32)
            sink = cpool.tile([128, T], f32)

            # ---- long-lived tensors (attention operands in bf16) ----
            qf = [lpool.tile([128, S], bf16, tag=f"qf{d}", name=f"qf{d}")
                  for d in range(4)]
            kf = lpool.tile([128, T], bf16, tag="kf")
            vpr = [lpool.tile([128, 16, 68], bf16, tag=f"vpr{g}", name=f"vpr{g}")
                   for g in range(2)]
            attn = [lpool.tile([128, S], bf16, tag=f"attn{p}", name=f"attn{p}")
                    for p in range(4)]
            wot = [lpool.tile([128, HID], bf16, tag=f"wot{p}", name=f"wot{p}")
                   for p in range(4)]
            ob0_bf = cpool.tile([128, 1], bf16)
            nc.vector.tensor_copy(ob0_bf[:], onesblk[:, 0:1])
            ob1_bf = cpool.tile([128, 1], bf16)
            nc.vector.tensor_copy(ob1_bf[:], onesblk[:, 1:2])
            bones_bf = cpool.tile([1, 64], bf16)
            nc.vector.tensor_copy(bones_bf[:], bones[:])
            ones_bf = cpool.tile([128, 16, 1], bf16)
            nc.vector.tensor_copy(ones_bf[:], ones_col[:])
            for g in range(2):
                nc.vector.tensor_copy(vpr[g][:, :, 64:65], ones_bf[:])

            PS = {}

            # norm+rope pipeline state per block
            blocks = []  # dicts with stage products

            def post_a(blk_ps, cos_ap, sin_ap, out_ap, W):
                d = {"ps": blk_ps, "cos": cos_ap, "sin": sin_ap,
                     "out": out_ap, "W": W}
                qcp = wk_pool.tile([128, W], bf16, tag="qcp", bufs=12,
                                   name="qcp")
                nc.vector.tensor_copy(qcp[:], blk_ps[:])
                d["qcp"] = qcp
                d["idx"] = len(blocks)
                if DEBUG and d["idx"] == 0:
                    nc.sync.dma_start(nb_dbg["qcp0"][:], qcp[:])
                blocks.append(d)

            def stage_s(d):
                W = d["W"]
                sq = wk_pool.tile([128, W], bf16, tag="sq", bufs=3, name="sq")
                # qcp is f32; Square downcasts to bf16 for the sum matmuls
                nc.scalar.activation(sq[:], d["qcp"][:], Act.Square)
                s0 = PS["p"].tile([1, W], f32, tag="s", bufs=2, name="psn0")
                nc.tensor.matmul(s0[:], ob0_bf[:], sq[:],
                                 start=True, stop=True)
                s1 = PS["p"].tile([1, W], f32, tag="s", bufs=2, name="psn1")
                nc.tensor.matmul(s1[:], ob1_bf[:], sq[:],
                                 start=True, stop=True)
                d["s0"], d["s1"] = s0, s1
                if DEBUG and d["idx"] == 0:
                    nc.sync.dma_start(nb_dbg["sq0"][:], sq[:])

            def stage_sqrt(d):
                W = d["W"]
                sc0 = wk_pool.tile([1, W], bf16, tag="sc0", bufs=3, name="sc0")
                sc1 = wk_pool.tile([1, W], bf16, tag="sc1", bufs=3, name="sc1")
                nc.scalar.activation(sc0[:], d["s0"][:], Act.Sqrt,
                                     scale=1.0 / 64.0, bias=eps_t[:])
                nc.scalar.activation(sc1[:], d["s1"][:], Act.Sqrt,
                                     scale=1.0 / 64.0, bias=eps_t[:])
                d["sc0"], d["sc1"] = sc0, sc1
                if DEBUG and d["idx"] == 0:
                    nc.sync.dma_start(nb_dbg["sc00"][:], sc0[:].bitcast(f32))

            def stage_b(d):
                W = d["W"]
                bps = PS["p"].tile([128, W], f32, tag="b", bufs=2, name="psb")
                nc.tensor.matmul(bps[0:64, :], bones_bf[:], d["sc0"][:],
                                 start=True, stop=True, tile_position=(0, 0))
                nc.tensor.matmul(bps[64:128, :], bones_bf[:], d["sc1"][:],
                                 start=True, stop=True, tile_position=(0, 64))
                d["bps"] = bps


            def stage_qn(d):
                W = d["W"]
                bc = wk_pool.tile([128, W], f32, tag="bc", bufs=3, name="bc")
                nc.vector.reciprocal_approx_fast(bc[:], d["bps"][:])
                qn = wk_pool.tile([128, W], f32r, tag="qn", bufs=3, name="qn")
                nc.vector.tensor_mul(qn[:], d["qcp"][:], bc[:])
                d["qn"] = qn
                if DEBUG and d["idx"] == 0:
                    nc.sync.dma_start(nb_dbg["bc0"][:], bc[:])
                    nc.sync.dma_start(nb_dbg["qn0"][:], qn[:].bitcast(f32))

            def stage_rot(d):
                W = d["W"]
                rot = PS["p"].tile([128, W], f32, tag="rot", bufs=2, name="psrot")
                nc.tensor.matmul(rot[:], prot[:], d["qn"][:],
                                 start=True, stop=True)
                d["rot"] = rot


            def stage_rope(d):
                W = d["W"]
                m1 = wk_pool.tile([128, W], f32, tag="m1", bufs=2, name="m1")
                nc.vector.tensor_mul(m1[:], d["qn"][:], d["cos"])
                m2 = wk_pool.tile([128, W], f32, tag="m2", bufs=2, name="m2")
                nc.vector.tensor_mul(m2[:], d["rot"][:], d["sin"])
                nc.vector.tensor_add(d["out"], m1[:], m2[:])
                if DEBUG and d["idx"] == 0:
                    nc.sync.dma_start(nb_dbg["m10"][:], m1[:])
                    nc.sync.dma_start(nb_dbg["m20"][:], m2[:])

            # ================= Phase 1: Q projection (h-outer) ==========
            ps1 = tc.tile_pool(name="ps1", bufs=1, space="PSUM")
            PS["p"] = ps1.__enter__()
            qps = [[PS["p"].tile([128, 512], f32, tag="proj", bufs=8,
                                 name=f"qps{s2}_{d}") for s2 in range(2)]
                   for d in range(4)]
            for h in range(16):
                xts = wk_pool.tile([128, S], f32r, tag="xts", bufs=3,
                                   name="xts")
                nc.sync.dma_start(
                    xts[:], xt_d[128 * h:128 * (h + 1), CTX:T].bitcast(f32r))
                wq = wk_pool.tile([128, 512], f32r, tag="wq", bufs=3,
                                  name="wq")
                nc.sync.dma_start(
                    wq[:], wqt_d[128 * h:128 * (h + 1), :].bitcast(f32r))
                for d in range(4):
                    for s2 in range(2):
                        nc.tensor.matmul(
                            qps[d][s2][:], wq[:, 128 * d:128 * (d + 1)],
                            xts[:, 512 * s2:512 * (s2 + 1)],
                            start=(h == 0), stop=(h == 15))
            for s2 in range(2):
                sl = slice(512 * s2, 512 * (s2 + 1))
                for d in range(4):
                    post_a(qps[d][s2], cosq[:, sl], sinq[:, sl],
                           qf[d][:, sl], 512)

            # ================= Phase 2: K/V projections =================
            kps = [PS["p"].tile([128, 512], f32, tag="proj", bufs=8,
                                    name=f"kps{tb}") for tb in range(4)]
            vps = [PS["p"].tile([128, 512], f32, tag="proj", bufs=8,
                                    name=f"vps{tb}") for tb in range(4)]
            for h in range(16):
                xtk = wk_pool.tile([128, T], f32r, tag="xtk", bufs=3,
                                   name="xtk")
                nc.sync.dma_start(
                    xtk[:], xt_d[128 * h:128 * (h + 1), :].bitcast(f32r))
                wkv = wk_pool.tile([128, 256], f32r, tag="wkv", bufs=3,
                                   name="wkv")
                nc.sync.dma_start(
                    wkv[:, 0:128], wkt_d[128 * h:128 * (h + 1), :].bitcast(f32r))
                nc.sync.dma_start(
                    wkv[:, 128:256],
                    wvt_d[128 * h:128 * (h + 1), :].bitcast(f32r))
                for tb in range(4):
                    xsl = xtk[:, 512 * tb:512 * (tb + 1)]
                    nc.tensor.matmul(kps[tb][:], wkv[:, 0:128], xsl,
                                     start=(h == 0), stop=(h == 15))
                    nc.tensor.matmul(vps[tb][:], wkv[:, 128:256], xsl,
                                     start=(h == 0), stop=(h == 15))
            nc.sync.dma_start(cosq[:], cosq_d[:])
            nc.sync.dma_start(sinq[:], sinq_d[:])
            nc.sync.dma_start(cosk[:], cosk_d[:])
            nc.sync.dma_start(sink[:], sink_d[:])
            for tb in range(4):
                sl = slice(512 * tb, 512 * (tb + 1))
                post_a(kps[tb], cosk[:, sl], sink[:, sl], kf[:, sl], 512)
            vcps = []
            for tb in range(4):
                vcp = wk_pool.tile([128, 512], f32, tag="vcp", bufs=4,
                                   name="vcp")
                nc.vector.tensor_copy(vcp[:], vps[tb][:])
                vcps.append(vcp)

            ps1.__exit__(None, None, None)
            ps2 = tc.tile_pool(name="ps2", bufs=1, space="PSUM")
            PS["p"] = ps2.__enter__()
            # ---- pipelined norm+rope stages: Q first (their post_a ran
            # during phase 2, so PE enters the window with zero stall), K last
            qorder = [blocks[d] for dd in range(4) for d in (dd, dd + 4)]
            blocks = qorder + blocks[8:]
            for d in blocks:
                stage_s(d)
            # V transposes keep PE busy while ACT does the sqrts
            tps = []
            for tb in range(4):
                for c in range(4):
                    tp = PS["p"].tile([128, 128], f32, tag="tp", bufs=2,
                                      name="tp")
                    nc.tensor.transpose(
                        tp[:], vcps[tb][:, 128 * c:128 * (c + 1)], ident[:])
                    tps.append(tp)
            for d in blocks:
                stage_sqrt(d)
            for i, tp in enumerate(tps):
                nc.vector.tensor_copy(vpr[0][:, i, 0:64], tp[:, 0:64])
                nc.vector.tensor_copy(vpr[1][:, i, 0:64], tp[:, 64:128])
            for i, d in enumerate(blocks):
                stage_b(d)
                if i >= 2:
                    stage_qn(blocks[i - 2])
                    stage_rot(blocks[i - 2])
                    stage_rope(blocks[i - 2])
            for d in blocks[-2:]:
                stage_qn(d)
                stage_rot(d)
                stage_rope(d)

            for p in range(4):
                nc.sync.dma_start(wot[p][:], wot_d[128 * p:128 * (p + 1), :])
            ps2.__exit__(None, None, None)
            ps3 = tc.tile_pool(name="ps3", bufs=1, space="PSUM")
            PS["p"] = ps3.__enter__()

            # ================= Phase 3: attention + Wo =================
            def emit_norm(hd):
                avp, pair, off, ssl = hd
                cs = wk_pool.tile([1, 512], f32r, tag="cs", bufs=2, name="cs")
                nc.vector.tensor_copy(cs[:], avp[64:65, :])
                bcp = PS["p"].tile([64, 512], f32, tag="bcp", bufs=1,
                                   name="bcp")
                nc.tensor.matmul(bcp[:], bones[:], cs[:], start=True,
                                 stop=True)
                rcb = wk_pool.tile([64, 512], f32, tag="rcb", bufs=2,
                                   name="rcb")
                nc.vector.reciprocal_approx_fast(rcb[:], bcp[:])
                nc.vector.tensor_mul(attn[pair][off:off + 64, ssl],
                                     avp[0:64, :], rcb[:])

            prev_heads = []
            for sb in range(2):
                ssl = slice(512 * sb, 512 * (sb + 1))
                for p4 in range(4):
                    # head pair (p4, p4+4): kv groups 0/1 at partition bases
                    # 0/64 -> QK pairs run concurrently in separate PE
                    # row-groups
                    qsA = qf[p4][0:64, ssl]
                    qsB = qf[p4][64:128, ssl]
                    avpA = PS["p"].tile([65, 512], f32, tag="avp", bufs=4,
                                        name="avpA")
                    avpB = PS["p"].tile([65, 512], f32, tag="avp", bufs=4,
                                        name="avpB")
                    expsA, expsB = [], []
                    for tcn in range(16):
                        tsl = slice(128 * tcn, 128 * (tcn + 1))
                        scpA = PS["p"].tile([128, 512], f32, tag="scp",
                                            bufs=3, name="scpA")
                        nc.tensor.matmul(scpA[:], kf[0:64, tsl], qsA,
                                         start=True, stop=True)
                        scpB = PS["p"].tile([128, 512], f32, tag="scp",
                                            bufs=3, name="scpB")
                        nc.tensor.matmul(scpB[:], kf[64:128, tsl], qsB,
                                         start=True, stop=True)
                        eA = wk_pool.tile([128, 512], bf16, tag="exp", bufs=8,
                                          name="expA")
                        nc.scalar.activation(eA[:], scpA[:], Act.Exp,
                                             scale=0.125)
                        eB = wk_pool.tile([128, 512], bf16, tag="exp", bufs=8,
                                          name="expB")
                        nc.scalar.activation(eB[:], scpB[:], Act.Exp,
                                             scale=0.125)
                        expsA.append(eA)
                        expsB.append(eB)
                        if tcn == 3:
                            for hd in prev_heads:
                                emit_norm(hd)
                            prev_heads = []
                        if tcn >= 1:
                            nc.tensor.matmul(avpA[:], vpr[0][:, tcn - 1, 0:65],
                                             expsA[tcn - 1][:],
                                             start=(tcn == 1), stop=False)
                            nc.tensor.matmul(avpB[:], vpr[1][:, tcn - 1, 0:65],
                                             expsB[tcn - 1][:],
                                             start=(tcn == 1), stop=False)
                    nc.tensor.matmul(avpA[:], vpr[0][:, 15, 0:65],
                                     expsA[15][:], start=False, stop=True)
                    nc.tensor.matmul(avpB[:], vpr[1][:, 15, 0:65],
                                     expsB[15][:], start=False, stop=True)
                    prev_heads = [(avpA, p4, 0, ssl), (avpB, p4, 64, ssl)]
                for hd in prev_heads:
                    emit_norm(hd)
                prev_heads = []
                # Wo for this s-block
                for st in range(4):
                    s0 = 512 * sb + 128 * st
                    osb = wk_pool.tile([128, HID], f32, tag="osb", bufs=2,
                                       name="osb")
                    for eb in range(4):
                        wops = PS["p"].tile([128, 512], f32, tag="scp",
                                            bufs=3, name="wops")
                        for p in range(4):
                            nc.tensor.matmul(
                                wops[:],
                                attn[p][:, s0:s0 + 128],
                                wot[p][:, 512 * eb:512 * (eb + 1)],
                                start=(p == 0), stop=(p == 3))
                        nc.vector.tensor_copy(
                            osb[:, 512 * eb:512 * (eb + 1)], wops[:])
                    nc.sync.dma_start(o_d[s0:s0 + 128, :], osb[:])
            if DEBUG:
                for d in range(4):
                    nc.sync.dma_start(qf_dbg[d], qf[d][:])
                    nc.sync.dma_start(attn_dbg[d], attn[d][:])
                nc.sync.dma_start(kf_dbg[:], kf[:])
                for g in range(2):
                    nc.sync.dma_start(vpr_dbg[g], vpr[g][:, :, 0:65])
            ps3.__exit__(None, None, None)

    nc.compile()
    return nc


def _host_inputs(hidden_states, target_context, cos, sin, Wq, Wk, Wv, Wo,
                 q_gamma, k_gamma):
    """Build the 8 per-core input maps."""
    f32 = np.float32
    P = np.zeros((128, 128), dtype=f32)
    for d in range(128):
        base, dd = (d // 64) * 64, d % 64
        if dd < 32:
            P[d, base + dd + 32] = -1.0
        else:
            P[d, base + dd - 32] = 1.0
    p_lhsT = np.ascontiguousarray(P.T)
    onesblk = np.zeros((128, 2), dtype=f32)
    onesblk[0:64, 0] = 1.0
    onesblk[64:128, 1] = 1.0
    ident = np.eye(128, dtype=f32)
    bones = np.ones((1, 64), dtype=f32)

    qg_rot = np.roll(q_gamma, -32)
    kg_rot = np.roll(k_gamma, -32)
    # head permutation: tile d holds heads (d, d+4) so that each head's
    # partition offset matches its kv-group offset in kf
    perm = np.concatenate(
        [np.arange(64 * h, 64 * h + 64) for h in (0, 4, 1, 5, 2, 6, 3, 7)])

    in_maps = []
    for core in range(8):
        b, hg = core // 4, core % 4
        if core % 4 == 0:
            xt = np.ascontiguousarray(
                np.concatenate([target_context[b], hidden_states[b]], 0).T
            ).astype(f32, copy=False)
            cosq = np.ascontiguousarray(
                np.tile((cos[b, CTX:] * q_gamma).T, (2, 1))).astype(f32)
            sinq = np.ascontiguousarray(
                np.tile((sin[b, CTX:] * qg_rot).T, (2, 1))).astype(f32)
            cosk = np.ascontiguousarray(
                np.tile((cos[b, :T] * k_gamma).T, (2, 1))).astype(f32)
            sink = np.ascontiguousarray(
                np.tile((sin[b, :T] * kg_rot).T, (2, 1))).astype(f32)
        wqt = np.ascontiguousarray(
            Wq[512 * hg:512 * (hg + 1), :][perm, :].T).astype(f32)
        wkt = np.ascontiguousarray(Wk[128 * hg:128 * (hg + 1), :].T).astype(f32)
        wvt = np.ascontiguousarray(Wv[128 * hg:128 * (hg + 1), :].T).astype(f32)
        wot = np.ascontiguousarray(
            Wo[:, 512 * hg:512 * (hg + 1)].T[perm, :]).astype(
                ml_dtypes.bfloat16)
        in_maps.append({
            "xt": xt, "wqt": wqt, "wkt": wkt, "wvt": wvt, "wot": wot,
            "cosq": cosq, "sinq": sinq, "cosk": cosk, "sink": sink,
            "prot": p_lhsT, "onesblk": onesblk, "ident": ident, "bones": bones,
        })
    return in_maps


def _run(in_maps, trace=False, trace_kwargs=None):
    from concourse.bass_utils import run_bass_kernel_spmd
    if "nc" not in _CACHE:
        _CACHE["nc"] = _build()
    kw = {}
    if trace:
        kw["trace"] = True
        if trace_kwargs:
            kw["trace_kwargs"] = trace_kwargs
    return run_bass_kernel_spmd(_CACHE["nc"], in_maps, list(range(8)), **kw)


def kernel(hidden_states, target_context, cos, sin, Wq, Wk, Wv, Wo,
           q_gamma, k_gamma, _trace=False):
    in_maps = _host_inputs(
        np.asarray(hidden_states, np.float32),
        np.asarray(target_context, np.float32),
        np.asarray(cos, np.float32), np.asarray(sin, np.float32),
        np.asarray(Wq, np.float32), np.asarray(Wk, np.float32),
        np.asarray(Wv, np.float32), np.asarray(Wo, np.float32),
        np.asarray(q_gamma, np.float32), np.asarray(k_gamma, np.float32))
    res = _run(in_maps, trace=_trace)
    out = np.zeros((B, S, HID), dtype=np.float32)
    for core in range(8):
        out[core // 4] += res.results[core]["o"]
    if _trace:
        return out, res
    return out


# revision 30
# speedup vs baseline: 1.0114x; 1.0114x over previous
"""Trainium2 Bass kernel for DFlashAttention (self-contained).

Sharding: 8 cores = 2 batches x 4 head-groups (tensor-parallel over heads).
Each core handles 8 q-heads / 2 kv-heads of one batch, computes a partial
output through Wo (input-dim sharded); host sums the 4 partials per batch.

Projections run as float32r matmuls (full PE speed, ~tf32 precision) with
activations/weights transposed on host. RMS-norm + RoPE run in the transposed
layout via PE tricks (ones-column sum matmuls, broadcast matmuls,
permutation-matrix rotate-half with signs and gamma folded into precomputed
cos/sin). The attention part (QK^T scores -> exp -> AV -> Wo) runs in bf16
operands with fp32 PSUM accumulation; softmax over t (the partition dim)
skips max-subtraction (scores bounded ~+-6) and gets its denominator free
from a ones-row appended to V, applied after AV via a broadcast matmul +
fast reciprocal. Emission is stage-pipelined so PE never stalls on ACT/DVE.
"""

import numpy as np
import ml_dtypes

NUM_HEADS = 32
NUM_KV_HEADS = 8
HEAD_DIM = 64
EPS = 1e-6
B, S, CTX, HID = 2, 1024, 1024, 2048
T = CTX + S  # 2048

_CACHE = {}


def _build():
    import concourse.bass as bass
    import concourse.mybir as mybir
    import concourse.tile as tile
    from concourse import bacc

    f32 = mybir.dt.float32
    f32r = mybir.dt.float32r
    bf16 = mybir.dt.bfloat16
    Act = mybir.ActivationFunctionType

    nc = bacc.Bacc("TRN2", target_bir_lowering=False, debug=False)

    # ---- DRAM I/O ----
    xt_d = nc.dram_tensor("xt", [HID, T], f32, kind="ExternalInput")
    wqt_d = nc.dram_tensor("wqt", [HID, 512], f32, kind="ExternalInput")
    wkt_d = nc.dram_tensor("wkt", [HID, 128], f32, kind="ExternalInput")
    wvt_d = nc.dram_tensor("wvt", [HID, 128], f32, kind="ExternalInput")
    wot_d = nc.dram_tensor("wot", [512, HID], bf16, kind="ExternalInput")
    cosq_d = nc.dram_tensor("cosq", [128, S], f32, kind="ExternalInput")
    sinq_d = nc.dram_tensor("sinq", [128, S], f32, kind="ExternalInput")
    cosk_d = nc.dram_tensor("cosk", [128, T], f32, kind="ExternalInput")
    sink_d = nc.dram_tensor("sink", [128, T], f32, kind="ExternalInput")
    prot_d = nc.dram_tensor("prot", [128, 128], f32, kind="ExternalInput")
    onesblk_d = nc.dram_tensor("onesblk", [128, 2], f32, kind="ExternalInput")
    ident_d = nc.dram_tensor("ident", [128, 128], f32, kind="ExternalInput")
    bones_d = nc.dram_tensor("bones", [1, 64], f32, kind="ExternalInput")
    o_d = nc.dram_tensor("o", [S, HID], f32, kind="ExternalOutput")
    DEBUG = _CACHE.get("debug", False)
    if DEBUG:
        qf_dbg = nc.dram_tensor("qf_dbg", [4, 128, S], bf16, kind="ExternalOutput")
        kf_dbg = nc.dram_tensor("kf_dbg", [128, T], bf16, kind="ExternalOutput")
        vpr_dbg = nc.dram_tensor("vpr_dbg", [2, 128, 16, 65], bf16, kind="ExternalOutput")
        attn_dbg = nc.dram_tensor("attn_dbg", [4, 128, S], bf16, kind="ExternalOutput")
        nb_dbg = {}
        for nm, shp, dt_ in [("qcp0", [128, 512], bf16), ("sq0", [128, 512], bf16),
                             ("s00", [1, 512], f32), ("sc00", [1, 512], f32),
                             ("b00", [64, 512], f32), ("bc0", [128, 512], f32),
                             ("qn0", [128, 512], f32), ("rot0", [128, 512], f32),
                             ("m10", [128, 512], f32), ("m20", [128, 512], f32)]:
            nb_dbg[nm] = nc.dram_tensor(nm, shp, dt_, kind="ExternalOutput")

    with tile.TileContext(nc) as tc:
        with (
            tc.tile_pool(name="consts", bufs=1) as cpool,
            tc.tile_pool(name="long", bufs=1) as lpool,
            tc.tile_pool(name="work", bufs=1) as wk_pool,
        ):
            # ---- constants ----
            prot = cpool.tile([128, 128], f32r)
            nc.sync.dma_start(prot[:], prot_d[:].bitcast(f32r))
            onesblk = cpool.tile([128, 2], f32r)
            nc.sync.dma_start(onesblk[:], onesblk_d[:].bitcast(f32r))
            ident = cpool.tile([128, 128], f32)
            nc.sync.dma_start(ident[:], ident_d[:])
            bones = cpool.tile([1, 64], f32r)
            nc.sync.dma_start(bones[:], bones_d[:].bitcast(f32r))
            eps_t = cpool.tile([1, 1], f32)
            nc.vector.memset(eps_t[:], EPS)
            ones_col = cpool.tile([128, 16, 1], f32)
            nc.vector.memset(ones_col[:], 1.0)
            cosq = cpool.tile([128, S], f32)
            sinq = cpool.tile([128, S], f32)
            cosk = cpool.tile([128, T], f32)
            sink = cpool.tile([128, T], f32)

            # ---- long-lived tensors (attention operands in bf16) ----
            qf = [lpool.tile([128, S], bf16, tag=f"qf{d}", name=f"qf{d}")
                  for d in range(4)]
            kf = lpool.tile([128, T], bf16, tag="kf")
            vpr = [lpool.tile([128, 16, 68], bf16, tag=f"vpr{g}", name=f"vpr{g}")
                   for g in range(2)]
            attn = [lpool.tile([128, S], bf16, tag=f"attn{p}", name=f"attn{p}")
                    for p in range(4)]
            wot = [lpool.tile([128, HID], bf16, tag=f"wot{p}", name=f"wot{p}")
                   for p in range(4)]
            ob0_bf = cpool.tile([128, 1], bf16)
            nc.vector.tensor_copy(ob0_bf[:], onesblk[:, 0:1])
            ob1_bf = cpool.tile([128, 1], bf16)
            nc.vector.tensor_copy(ob1_bf[:], onesblk[:, 1:2])
            bones_bf = cpool.tile([1, 64], bf16)
            nc.vector.tensor_copy(bones_bf[:], bones[:])
            ones_bf = cpool.tile([128, 16, 1], bf16)
            nc.vector.tensor_copy(ones_bf[:], ones_col[:])
            for g in range(2):
                nc.vector.tensor_copy(vpr[g][:, :, 64:65], ones_bf[:])

            PS = {}

            # norm+rope pipeline state per block
            blocks = []  # dicts with stage products

            def post_a(blk_ps, cos_ap, sin_ap, out_ap, W):
                d = {"ps": blk_ps, "cos": cos_ap, "sin": sin_ap,
                     "out": out_ap, "W": W}
                qcp = wk_pool.tile([128, W], bf16, tag="qcp", bufs=12,
                                   name="qcp")
                nc.vector.tensor_copy(qcp[:], blk_ps[:])
                d["qcp"] = qcp
                d["idx"] = len(blocks)
                if DEBUG and d["idx"] == 0:
                    nc.sync.dma_start(nb_dbg["qcp0"][:], qcp[:])
                blocks.append(d)

            def stage_s(d):
                W = d["W"]
                sq = wk_pool.tile([128, W], bf16, tag="sq", bufs=3, name="sq")
                # qcp is f32; Square downcasts to bf16 for the sum matmuls
                nc.scalar.activation(sq[:], d["qcp"][:], Act.Square)
                s0 = PS["p"].tile([1, W], f32, tag="s", bufs=2, name="psn0")
                nc.tensor.matmul(s0[:], ob0_bf[:], sq[:],
                                 start=True, stop=True)
                s1 = PS["p"].tile([1, W], f32, tag="s", bufs=2, name="psn1")
                nc.tensor.matmul(s1[:], ob1_bf[:], sq[:],
                                 start=True, stop=True)
                d["s0"], d["s1"] = s0, s1
                if DEBUG and d["idx"] == 0:
                    nc.sync.dma_start(nb_dbg["sq0"][:], sq[:])

            def stage_sqrt(d):
                W = d["W"]
                sc0 = wk_pool.tile([1, W], bf16, tag="sc0", bufs=3, name="sc0")
                sc1 = wk_pool.tile([1, W], bf16, tag="sc1", bufs=3, name="sc1")
                nc.scalar.activation(sc0[:], d["s0"][:], Act.Sqrt,
                                     scale=1.0 / 64.0, bias=eps_t[:])
                nc.scalar.activation(sc1[:], d["s1"][:], Act.Sqrt,
                                     scale=1.0 / 64.0, bias=eps_t[:])
                d["sc0"], d["sc1"] = sc0, sc1
                if DEBUG and d["idx"] == 0:
                    nc.sync.dma_start(nb_dbg["sc00"][:], sc0[:].bitcast(f32))

            def stage_b(d):
                W = d["W"]
                bps = PS["p"].tile([128, W], f32, tag="b", bufs=2, name="psb")
                nc.tensor.matmul(bps[0:64, :], bones_bf[:], d["sc0"][:],
                                 start=True, stop=True, tile_position=(0, 0))
                nc.tensor.matmul(bps[64:128, :], bones_bf[:], d["sc1"][:],
                                 start=True, stop=True, tile_position=(0, 64))
                d["bps"] = bps


            def stage_qn(d):
                W = d["W"]
                bc = wk_pool.tile([128, W], f32, tag="bc", bufs=3, name="bc")
                nc.vector.reciprocal_approx_fast(bc[:], d["bps"][:])
                qn = wk_pool.tile([128, W], f32r, tag="qn", bufs=3, name="qn")
                nc.vector.tensor_mul(qn[:], d["qcp"][:], bc[:])
                d["qn"] = qn
                if DEBUG and d["idx"] == 0:
                    nc.sync.dma_start(nb_dbg["bc0"][:], bc[:])
                    nc.sync.dma_start(nb_dbg["qn0"][:], qn[:].bitcast(f32))

            def stage_rot(d):
                W = d["W"]
                rot = PS["p"].tile([128, W], f32, tag="rot", bufs=2, name="psrot")
                nc.tensor.matmul(rot[:], prot[:], d["qn"][:],
                                 start=True, stop=True)
                d["rot"] = rot


            def stage_rope(d):
                W = d["W"]
                m1 = wk_pool.tile([128, W], f32, tag="m1", bufs=2, name="m1")
                nc.vector.tensor_mul(m1[:], d["qn"][:], d["cos"])
                m2 = wk_pool.tile([128, W], f32, tag="m2", bufs=2, name="m2")
                nc.vector.tensor_mul(m2[:], d["rot"][:], d["sin"])
                nc.vector.tensor_add(d["out"], m1[:], m2[:])
                if DEBUG and d["idx"] == 0:
                    nc.sync.dma_start(nb_dbg["m10"][:], m1[:])
                    nc.sync.dma_start(nb_dbg["m20"][:], m2[:])

            # ================= Phase 1: Q projection (h-outer) ==========
            ps1 = tc.tile_pool(name="ps1", bufs=1, space="PSUM")
            PS["p"] = ps1.__enter__()
            qps = [[PS["p"].tile([128, 512], f32, tag="proj", bufs=8,
                                 name=f"qps{s2}_{d}") for s2 in range(2)]
                   for d in range(4)]
            for h in range(16):
                xts = wk_pool.tile([128, S], f32r, tag="xts", bufs=3,
                                   name="xts")
                nc.sync.dma_start(
                    xts[:], xt_d[128 * h:128 * (h + 1), CTX:T].bitcast(f32r))
                wq = wk_pool.tile([128, 512], f32r, tag="wq", bufs=3,
                                  name="wq")
                nc.sync.dma_start(
                    wq[:], wqt_d[128 * h:128 * (h + 1), :].bitcast(f32r))
                for d in range(4):
                    for s2 in range(2):
                        nc.tensor.matmul(
                            qps[d][s2][:], wq[:, 128 * d:128 * (d + 1)],
                            xts[:, 512 * s2:512 * (s2 + 1)],
                            start=(h == 0), stop=(h == 15))
            for s2 in range(2):
                sl = slice(512 * s2, 512 * (s2 + 1))
                for d in range(4):
                    post_a(qps[d][s2], cosq[:, sl], sinq[:, sl],
                           qf[d][:, sl], 512)

            # ================= Phase 2: K/V projections =================
            kps = [PS["p"].tile([128, 512], f32, tag="proj", bufs=8,
                                    name=f"kps{tb}") for tb in range(4)]
            vps = [PS["p"].tile([128, 512], f32, tag="proj", bufs=8,
                                    name=f"vps{tb}") for tb in range(4)]
            for h in range(16):
                xtk = wk_pool.tile([128, T], f32r, tag="xtk", bufs=3,
                                   name="xtk")
                nc.sync.dma_start(
                    xtk[:], xt_d[128 * h:128 * (h + 1), :].bitcast(f32r))
                wkv = wk_pool.tile([128, 256], f32r, tag="wkv", bufs=3,
                                   name="wkv")
                nc.sync.dma_start(
                    wkv[:, 0:128], wkt_d[128 * h:128 * (h + 1), :].bitcast(f32r))
                nc.sync.dma_start(
                    wkv[:, 128:256],
                    wvt_d[128 * h:128 * (h + 1), :].bitcast(f32r))
                for tb in range(4):
                    xsl = xtk[:, 512 * tb:512 * (tb + 1)]
                    nc.tensor.matmul(kps[tb][:], wkv[:, 0:128], xsl,
                                     start=(h == 0), stop=(h == 15))
                    nc.tensor.matmul(vps[tb][:], wkv[:, 128:256], xsl,
                                     start=(h == 0), stop=(h == 15))
            nc.sync.dma_start(cosq[:], cosq_d[:])
            nc.sync.dma_start(sinq[:], sinq_d[:])
            nc.sync.dma_start(cosk[:], cosk_d[:])
            nc.sync.dma_start(sink[:], sink_d[:])
            for tb in range(4):
                sl = slice(512 * tb, 512 * (tb + 1))
                post_a(kps[tb], cosk[:, sl], sink[:, sl], kf[:, sl], 512)
            vcps = []
            for tb in range(4):
                vcp = wk_pool.tile([128, 512], f32, tag="vcp", bufs=4,
                                   name="vcp")
                nc.vector.tensor_copy(vcp[:], vps[tb][:])
                vcps.append(vcp)

            ps1.__exit__(None, None, None)
            ps2 = tc.tile_pool(name="ps2", bufs=1, space="PSUM")
            PS["p"] = ps2.__enter__()
            # ---- pipelined norm+rope stages: Q first (their post_a ran
            # during phase 2, so PE enters the window with zero stall), K last
            qorder = [blocks[d] for dd in range(4) for d in (dd, dd + 4)]
            blocks = qorder + blocks[8:]
            for d in blocks:
                stage_s(d)
            # V transposes keep PE busy while ACT does the sqrts
            tps = []
            for tb in range(4):
                for c in range(4):
                    tp = PS["p"].tile([128, 128], f32, tag="tp", bufs=2,
                                      name="tp")
                    nc.tensor.transpose(
                        tp[:], vcps[tb][:, 128 * c:128 * (c + 1)], ident[:])
                    tps.append(tp)
            for d in blocks:
                stage_sqrt(d)
            for i, tp in enumerate(tps):
                nc.vector.tensor_copy(vpr[0][:, i, 0:64], tp[:, 0:64])
                nc.vector.tensor_copy(vpr[1][:, i, 0:64], tp[:, 64:128])
            for i, d in enumerate(blocks):
                stage_b(d)
                if i >= 2:
                    stage_qn(blocks[i - 2])
                    stage_rot(blocks[i - 2])
                    stage_rope(blocks[i - 2])
            for d in blocks[-2:]:
                stage_qn(d)
                stage_rot(d)
                stage_rope(d)

            for p in range(4):
                nc.sync.dma_start(wot[p][:], wot_d[128 * p:128 * (p + 1), :])
            ps2.__exit__(None, None, None)
            ps3 = tc.tile_pool(name="ps3", bufs=1, space="PSUM")
            PS["p"] = ps3.__enter__()

            # ================= Phase 3: attention + Wo =================
            def emit_norm(hd):
                avp, pair, off, ssl = hd
                cs = wk_pool.tile([1, 512], f32r, tag="cs", bufs=2, name="cs")
                nc.vector.tensor_copy(cs[:], avp[64:65, :])
                bcp = PS["p"].tile([64, 512], f32, tag="bcp", bufs=1,
                                   name="bcp")
                nc.tensor.matmul(bcp[:], bones[:], cs[:], start=True,
                                 stop=True)
                rcb = wk_pool.tile([64, 512], f32, tag="rcb", bufs=2,
                                   name="rcb")
                nc.vector.reciprocal_approx_fast(rcb[:], bcp[:])
                nc.vector.tensor_mul(attn[pair][off:off + 64, ssl],
                                     avp[0:64, :], rcb[:])

            prev_heads = []
            for sb in range(2):
                ssl = slice(512 * sb, 512 * (sb + 1))
                for p4 in range(4):
                    # head pair (p4, p4+4): kv groups 0/1 at partition bases
                    # 0/64 -> QK pairs run concurrently in separate PE
                    # row-groups
                    qsA = qf[p4][0:64, ssl]
                    qsB = qf[p4][64:128, ssl]
                    avpA = PS["p"].tile([65, 512], f32, tag="avp", bufs=4,
                                        name="avpA")
                    avpB = PS["p"].tile([65, 512], f32, tag="avp", bufs=4,
                                        name="avpB")
                    expsA, expsB = [], []
                    for tcn in range(16):
                        tsl = slice(128 * tcn, 128 * (tcn + 1))
                        scpA = PS["p"].tile([128, 512], f32, tag="scp",
                                            bufs=3, name="scpA")
                        nc.tensor.matmul(scpA[:], kf[0:64, tsl], qsA,
                                         start=True, stop=True)
                        scpB = PS["p"].tile([128, 512], f32, tag="scp",
                                            bufs=3, name="scpB")
                        nc.tensor.matmul(scpB[:], kf[64:128, tsl], qsB,
                                         start=True, stop=True)
                        eA = wk_pool.tile([128, 512], bf16, tag="exp", bufs=8,
                                          name="expA")
                        nc.scalar.activation(eA[:], scpA[:], Act.Exp,
                                             scale=0.125)
                        eB = wk_pool.tile([128, 512], bf16, tag="exp", bufs=8,
                                          name="expB")
                        nc.scalar.activation(eB[:], scpB[:], Act.Exp,
                                             scale=0.125)
                        expsA.append(eA)
                        expsB.append(eB)
                        if tcn == 3:
                            for hd in prev_heads:
                                emit_norm(hd)
                            prev_heads = []
                        if tcn >= 1:
                            nc.tensor.matmul(avpA[:], vpr[0][:, tcn - 1, 0:65],
                                             expsA[tcn - 1][:],
                                             start=(tcn == 1), stop=False)
                            nc.tensor.matmul(avpB[:], vpr[1][:, tcn - 1, 0:65],
                                             expsB[tcn - 1][:],
                                             start=(tcn == 1), stop=False)
                    nc.tensor.matmul(avpA[:], vpr[0][:, 15, 0:65],
                                     expsA[15][:], start=False, stop=True)
                    nc.tensor.matmul(avpB[:], vpr[1][:, 15, 0:65],
                                     expsB[15][:], start=False, stop=True)
                    prev_heads = [(avpA, p4, 0, ssl), (avpB, p4, 64, ssl)]
                for hd in prev_heads:
                    emit_norm(hd)
                prev_heads = []
                # Wo for this s-block
                for st in range(4):
                    s0 = 512 * sb + 128 * st
                    osb = wk_pool.tile([128, HID], f32, tag="osb", bufs=2,
                                       name="osb")
                    for eb in range(4):
                        wops = PS["p"].tile([128, 512], f32, tag="scp",
                                            bufs=3, name="wops")
                        for p in range(4):
                            nc.tensor.matmul(
                                wops[:],
                                attn[p][:, s0:s0 + 128],
                                wot[p][:, 512 * eb:512 * (eb + 1)],
                                start=(p == 0), stop=(p == 3))
                        nc.vector.tensor_copy(
                            osb[:, 512 * eb:512 * (eb + 1)], wops[:])
                    nc.sync.dma_start(o_d[s0:s0 + 128, :], osb[:])
            if DEBUG:
                for d in range(4):
                    nc.sync.dma_start(qf_dbg[d], qf[d][:])
                    nc.sync.dma_start(attn_dbg[d], attn[d][:])
                nc.sync.dma_start(kf_dbg[:], kf[:])
                for g in range(2):
                    nc.sync.dma_start(vpr_dbg[g], vpr[g][:, :, 0:65])
            ps3.__exit__(None, None, None)

    nc.compile()
    return nc


def _host_inputs(hidden_states, target_context, cos, sin, Wq, Wk, Wv, Wo,
                 q_gamma, k_gamma):
    """Build the 8 per-core input maps."""
    f32 = np.float32
    P = np.zeros((128, 128), dtype=f32)
    for d in range(128):
        base, dd = (d // 64) * 64, d % 64
        if dd < 32:
            P[d, base + dd + 32] = -1.0
        else:
            P[d, base + dd - 32] = 1.0
    p_lhsT = np.ascontiguousarray(P.T)
    onesblk = np.zeros((128, 2), dtype=f32)
    onesblk[0:64, 0] = 1.0
    onesblk[64:128, 1] = 1.0
    ident = np.eye(128, dtype=f32)
    bones = np.ones((1, 64), dtype=f32)

    qg_rot = np.roll(q_gamma, -32)
    kg_rot = np.roll(k_gamma, -32)
    # head permutation: tile d holds heads (d, d+4) so that each head's
    # partition offset matches its kv-group offset in kf
    perm = np.concatenate(
        [np.arange(64 * h, 64 * h + 64) for h in (0, 4, 1, 5, 2, 6, 3, 7)])

    in_maps = []
    for core in range(8):
        b, hg = core // 4, core % 4
        if core % 4 == 0:
            xt = np.ascontiguousarray(
                np.concatenate([target_context[b], hidden_states[b]], 0).T
            ).astype(f32, copy=False)
            cosq = np.ascontiguousarray(
                np.tile((cos[b, CTX:] * q_gamma).T, (2, 1))).astype(f32)
            sinq = np.ascontiguousarray(
                np.tile((sin[b, CTX:] * qg_rot).T, (2, 1))).astype(f32)
            cosk = np.ascontiguousarray(
                np.tile((cos[b, :T] * k_gamma).T, (2, 1))).astype(f32)
            sink = np.ascontiguousarray(
                np.tile((sin[b, :T] * kg_rot).T, (2, 1))).astype(f32)
        wqt = np.ascontiguousarray(
            Wq[512 * hg:512 * (hg + 1), :][perm, :].T).astype(f32)
        wkt = np.ascontiguousarray(Wk[128 * hg:128 * (hg + 1), :].T).astype(f32)
        wvt = np.ascontiguousarray(Wv[128 * hg:128 * (hg + 1), :].T).astype(f32)
        wot = np.ascontiguousarray(
            Wo[:, 512 * hg:512 * (hg + 1)].T[perm, :]).astype(
                ml_dtypes.bfloat16)
        in_maps.append({
            "xt": xt, "wqt": wqt, "wkt": wkt, "wvt": wvt, "wot": wot,
            "cosq": cosq, "sinq": sinq, "cosk": cosk, "sink": sink,
            "prot": p_lhsT, "onesblk": onesblk, "ident": ident, "bones": bones,
        })
    return in_maps


def _run(in_maps, trace=False, trace_kwargs=None):
    from concourse.bass_utils import run_bass_kernel_spmd
    if "nc" not in _CACHE:
        _CACHE["nc"] = _build()
    kw = {}
    if trace:
        kw["trace"] = True
        if trace_kwargs:
            kw["trace_kwargs"] = trace_kwargs
    return run_bass_kernel_spmd(_CACHE["nc"], in_maps, list(range(8)), **kw)


def kernel(hidden_states, target_context, cos, sin, Wq, Wk, Wv, Wo,
           q_gamma, k_gamma, _trace=False):
    in_maps = _host_inputs(
        np.asarray(hidden_states, np.float32),
        np.asarray(target_context, np.float32),
        np.asarray(cos, np.float32), np.asarray(sin, np.float32),
        np.asarray(Wq, np.float32), np.asarray(Wk, np.float32),
        np.asarray(Wv, np.float32), np.asarray(Wo, np.float32),
        np.asarray(q_gamma, np.float32), np.asarray(k_gamma, np.float32))
    res = _run(in_maps, trace=_trace)
    out = np.zeros((B, S, HID), dtype=np.float32)
    for core in range(8):
        out[core // 4] += res.results[core]["o"]
    if _trace:
        return out, res
    return out


# revision 32
# speedup vs baseline: 1.0381x; 1.0264x over previous
"""Trainium2 Bass kernel for DFlashAttention (self-contained).

Sharding: 8 cores = 2 batches x 4 head-groups (tensor-parallel over heads).
Each core handles 8 q-heads / 2 kv-heads of one batch, computes a partial
output through Wo (input-dim sharded); host sums the 4 partials per batch.

Projections run as float32r matmuls (full PE speed, ~tf32 precision) with
activations/weights transposed on host. RMS-norm + RoPE run in the transposed
layout via PE tricks (ones-column sum matmuls, broadcast matmuls,
permutation-matrix rotate-half with signs and gamma folded into precomputed
cos/sin). The attention part (QK^T scores -> exp -> AV -> Wo) runs in bf16
operands with fp32 PSUM accumulation; softmax over t (the partition dim)
skips max-subtraction (scores bounded ~+-6) and gets its denominator free
from a ones-row appended to V, applied after AV via a broadcast matmul +
fast reciprocal. Emission is stage-pipelined so PE never stalls on ACT/DVE.
"""

import numpy as np
import ml_dtypes

NUM_HEADS = 32
NUM_KV_HEADS = 8
HEAD_DIM = 64
EPS = 1e-6
B, S, CTX, HID = 2, 1024, 1024, 2048
T = CTX + S  # 2048

_CACHE = {}


def _build():
    import concourse.bass as bass
    import concourse.mybir as mybir
    import concourse.tile as tile
    from concourse import bacc

    f32 = mybir.dt.float32
    f32r = mybir.dt.float32r
    bf16 = mybir.dt.bfloat16
    Act = mybir.ActivationFunctionType

    nc = bacc.Bacc("TRN2", target_bir_lowering=False, debug=False)

    # ---- DRAM I/O ----
    xt_d = nc.dram_tensor("xt", [HID, T], f32, kind="ExternalInput")
    wqt_d = nc.dram_tensor("wqt", [HID, 512], f32, kind="ExternalInput")
    wkt_d = nc.dram_tensor("wkt", [HID, 128], f32, kind="ExternalInput")
    wvt_d = nc.dram_tensor("wvt", [HID, 128], f32, kind="ExternalInput")
    wot_d = nc.dram_tensor("wot", [512, HID], bf16, kind="ExternalInput")
    cosq_d = nc.dram_tensor("cosq", [128, S], f32, kind="ExternalInput")
    sinq_d = nc.dram_tensor("sinq", [128, S], f32, kind="ExternalInput")
    cosk_d = nc.dram_tensor("cosk", [128, T], f32, kind="ExternalInput")
    sink_d = nc.dram_tensor("sink", [128, T], f32, kind="ExternalInput")
    prot_d = nc.dram_tensor("prot", [128, 128], f32, kind="ExternalInput")
    onesblk_d = nc.dram_tensor("onesblk", [128, 2], f32, kind="ExternalInput")
    ident_d = nc.dram_tensor("ident", [128, 128], f32, kind="ExternalInput")
    bones_d = nc.dram_tensor("bones", [1, 64], f32, kind="ExternalInput")
    o_d = nc.dram_tensor("o", [S, HID], f32, kind="ExternalOutput")
    DEBUG = _CACHE.get("debug", False)
    if DEBUG:
        qf_dbg = nc.dram_tensor("qf_dbg", [4, 128, S], bf16, kind="ExternalOutput")
        kf_dbg = nc.dram_tensor("kf_dbg", [128, T], bf16, kind="ExternalOutput")
        vpr_dbg = nc.dram_tensor("vpr_dbg", [2, 128, 16, 65], bf16, kind="ExternalOutput")
        attn_dbg = nc.dram_tensor("attn_dbg", [4, 128, S], bf16, kind="ExternalOutput")
        nb_dbg = {}
        for nm, shp, dt_ in [("qcp0", [128, 512], bf16), ("sq0", [128, 512], bf16),
                             ("s00", [1, 512], f32), ("sc00", [1, 512], f32),
                             ("b00", [64, 512], f32), ("bc0", [128, 512], f32),
                             ("qn0", [128, 512], f32), ("rot0", [128, 512], f32),
                             ("m10", [128, 512], f32), ("m20", [128, 512], f32)]:
            nb_dbg[nm] = nc.dram_tensor(nm, shp, dt_, kind="ExternalOutput")

    with tile.TileContext(nc) as tc:
        with (
            tc.tile_pool(name="consts", bufs=1) as cpool,
            tc.tile_pool(name="long", bufs=1) as lpool,
            tc.tile_pool(name="work", bufs=1) as wk_pool,
        ):
            # ---- constants ----
            prot = cpool.tile([128, 128], f32r)
            nc.sync.dma_start(prot[:], prot_d[:].bitcast(f32r))
            onesblk = cpool.tile([128, 2], f32r)
            nc.sync.dma_start(onesblk[:], onesblk_d[:].bitcast(f32r))
            ident = cpool.tile([128, 128], f32)
            nc.sync.dma_start(ident[:], ident_d[:])
            bones = cpool.tile([1, 64], f32r)
            nc.sync.dma_start(bones[:], bones_d[:].bitcast(f32r))
            eps_t = cpool.tile([1, 1], f32)
            nc.vector.memset(eps_t[:], EPS)
            ones_col = cpool.tile([128, 16, 1], f32)
            nc.vector.memset(ones_col[:], 1.0)
            cosq = cpool.tile([128, S], f32)
            sinq = cpool.tile([128, S], f32)
            cosk = cpool.tile([128, T], f32)
            sink = cpool.tile([128, T], f32)

            # ---- long-lived tensors (attention operands in bf16) ----
            qf = [lpool.tile([128, S], bf16, tag=f"qf{d}", name=f"qf{d}")
                  for d in range(4)]
            kf = lpool.tile([128, T], bf16, tag="kf")
            vpr = [lpool.tile([128, 16, 68], bf16, tag=f"vpr{g}", name=f"vpr{g}")
                   for g in range(2)]
            attn = [lpool.tile([128, S], bf16, tag=f"attn{p}", name=f"attn{p}")
                    for p in range(4)]
            wot = [lpool.tile([128, HID], bf16, tag=f"wot{p}", name=f"wot{p}")
                   for p in range(4)]
            ob0_bf = cpool.tile([128, 1], bf16)
            nc.vector.tensor_copy(ob0_bf[:], onesblk[:, 0:1])
            ob1_bf = cpool.tile([128, 1], bf16)
            nc.vector.tensor_copy(ob1_bf[:], onesblk[:, 1:2])
            bones_bf = cpool.tile([1, 64], bf16)
            nc.vector.tensor_copy(bones_bf[:], bones[:])
            ones_bf = cpool.tile([128, 16, 1], bf16)
            nc.vector.tensor_copy(ones_bf[:], ones_col[:])
            for g in range(2):
                nc.vector.tensor_copy(vpr[g][:, :, 64:65], ones_bf[:])

            PS = {}

            # norm+rope pipeline state per block
            blocks = []  # dicts with stage products

            def post_a(blk_ps, cos_ap, sin_ap, out_ap, W):
                d = {"ps": blk_ps, "cos": cos_ap, "sin": sin_ap,
                     "out": out_ap, "W": W}
                qcp = wk_pool.tile([128, W], bf16, tag="qcp", bufs=12,
                                   name="qcp")
                nc.vector.tensor_copy(qcp[:], blk_ps[:])
                d["qcp"] = qcp
                d["idx"] = len(blocks)
                if DEBUG and d["idx"] == 0:
                    nc.sync.dma_start(nb_dbg["qcp0"][:], qcp[:])
                blocks.append(d)

            def stage_s(d):
                W = d["W"]
                sq = wk_pool.tile([128, W], bf16, tag="sq", bufs=3, name="sq")
                # qcp is f32; Square downcasts to bf16 for the sum matmuls
                nc.scalar.activation(sq[:], d["qcp"][:], Act.Square)
                s0 = PS["p"].tile([1, W], f32, tag="s", bufs=2, name="psn0")
                nc.tensor.matmul(s0[:], ob0_bf[:], sq[:],
                                 start=True, stop=True)
                s1 = PS["p"].tile([1, W], f32, tag="s", bufs=2, name="psn1")
                nc.tensor.matmul(s1[:], ob1_bf[:], sq[:],
                                 start=True, stop=True)
                d["s0"], d["s1"] = s0, s1
                if DEBUG and d["idx"] == 0:
                    nc.sync.dma_start(nb_dbg["sq0"][:], sq[:])

            def stage_sqrt(d):
                W = d["W"]
                sc0 = wk_pool.tile([1, W], bf16, tag="sc0", bufs=3, name="sc0")
                sc1 = wk_pool.tile([1, W], bf16, tag="sc1", bufs=3, name="sc1")
                nc.scalar.activation(sc0[:], d["s0"][:], Act.Sqrt,
                                     scale=1.0 / 64.0, bias=eps_t[:])
                nc.scalar.activation(sc1[:], d["s1"][:], Act.Sqrt,
                                     scale=1.0 / 64.0, bias=eps_t[:])
                d["sc0"], d["sc1"] = sc0, sc1
                if DEBUG and d["idx"] == 0:
                    nc.sync.dma_start(nb_dbg["sc00"][:], sc0[:].bitcast(f32))

            def stage_b(d):
                W = d["W"]
                bps = PS["p"].tile([128, W], f32, tag="b", bufs=2, name="psb")
                nc.tensor.matmul(bps[0:64, :], bones_bf[:], d["sc0"][:],
                                 start=True, stop=True, tile_position=(0, 0))
                nc.tensor.matmul(bps[64:128, :], bones_bf[:], d["sc1"][:],
                                 start=True, stop=True, tile_position=(0, 64))
                d["bps"] = bps


            def stage_qn(d):
                W = d["W"]
                bc = wk_pool.tile([128, W], f32, tag="bc", bufs=3, name="bc")
                nc.vector.reciprocal_approx_fast(bc[:], d["bps"][:])
                qn = wk_pool.tile([128, W], f32r, tag="qn", bufs=3, name="qn")
                nc.vector.tensor_mul(qn[:], d["qcp"][:], bc[:])
                d["qn"] = qn
                if DEBUG and d["idx"] == 0:
                    nc.sync.dma_start(nb_dbg["bc0"][:], bc[:])
                    nc.sync.dma_start(nb_dbg["qn0"][:], qn[:].bitcast(f32))

            def stage_rot(d):
                W = d["W"]
                rot = PS["p"].tile([128, W], f32, tag="rot", bufs=2, name="psrot")
                nc.tensor.matmul(rot[:], prot[:], d["qn"][:],
                                 start=True, stop=True)
                d["rot"] = rot


            def stage_rope(d):
                W = d["W"]
                m1 = wk_pool.tile([128, W], f32, tag="m1", bufs=2, name="m1")
                nc.vector.tensor_mul(m1[:], d["qn"][:], d["cos"])
                m2 = wk_pool.tile([128, W], f32, tag="m2", bufs=2, name="m2")
                nc.vector.tensor_mul(m2[:], d["rot"][:], d["sin"])
                nc.vector.tensor_add(d["out"], m1[:], m2[:])
                if DEBUG and d["idx"] == 0:
                    nc.sync.dma_start(nb_dbg["m10"][:], m1[:])
                    nc.sync.dma_start(nb_dbg["m20"][:], m2[:])

            # ================= Phase 1: Q projection (h-outer) ==========
            ps1 = tc.tile_pool(name="ps1", bufs=1, space="PSUM")
            PS["p"] = ps1.__enter__()
            qps = [[PS["p"].tile([128, 512], f32, tag="proj", bufs=8,
                                 name=f"qps{s2}_{d}") for s2 in range(2)]
                   for d in range(4)]
            for h in range(16):
                xts = wk_pool.tile([128, S], f32r, tag="xts", bufs=3,
                                   name="xts")
                nc.sync.dma_start(
                    xts[:], xt_d[128 * h:128 * (h + 1), CTX:T].bitcast(f32r))
                wq = wk_pool.tile([128, 512], f32r, tag="wq", bufs=3,
                                  name="wq")
                nc.sync.dma_start(
                    wq[:], wqt_d[128 * h:128 * (h + 1), :].bitcast(f32r))
                for d in range(4):
                    for s2 in range(2):
                        nc.tensor.matmul(
                            qps[d][s2][:], wq[:, 128 * d:128 * (d + 1)],
                            xts[:, 512 * s2:512 * (s2 + 1)],
                            start=(h == 0), stop=(h == 15))
            for s2 in range(2):
                sl = slice(512 * s2, 512 * (s2 + 1))
                for d in range(4):
                    post_a(qps[d][s2], cosq[:, sl], sinq[:, sl],
                           qf[d][:, sl], 512)

            # ================= Phase 2: K/V projections =================
            kps = [PS["p"].tile([128, 512], f32, tag="proj", bufs=8,
                                    name=f"kps{tb}") for tb in range(4)]
            vps = [PS["p"].tile([128, 512], f32, tag="proj", bufs=8,
                                    name=f"vps{tb}") for tb in range(4)]
            for h in range(16):
                xtk = wk_pool.tile([128, T], f32r, tag="xtk", bufs=3,
                                   name="xtk")
                nc.sync.dma_start(
                    xtk[:], xt_d[128 * h:128 * (h + 1), :].bitcast(f32r))
                wkv = wk_pool.tile([128, 256], f32r, tag="wkv", bufs=3,
                                   name="wkv")
                nc.sync.dma_start(
                    wkv[:, 0:128], wkt_d[128 * h:128 * (h + 1), :].bitcast(f32r))
                nc.sync.dma_start(
                    wkv[:, 128:256],
                    wvt_d[128 * h:128 * (h + 1), :].bitcast(f32r))
                for tb in range(4):
                    xsl = xtk[:, 512 * tb:512 * (tb + 1)]
                    nc.tensor.matmul(kps[tb][:], wkv[:, 0:128], xsl,
                                     start=(h == 0), stop=(h == 15))
                    nc.tensor.matmul(vps[tb][:], wkv[:, 128:256], xsl,
                                     start=(h == 0), stop=(h == 15))
            nc.sync.dma_start(cosq[:], cosq_d[:])
            nc.sync.dma_start(sinq[:], sinq_d[:])
            nc.sync.dma_start(cosk[:], cosk_d[:])
            nc.sync.dma_start(sink[:], sink_d[:])
            for tb in range(4):
                sl = slice(512 * tb, 512 * (tb + 1))
                post_a(kps[tb], cosk[:, sl], sink[:, sl], kf[:, sl], 512)
            vcps = []
            for tb in range(4):
                vcp = wk_pool.tile([128, 512], f32, tag="vcp", bufs=4,
                                   name="vcp")
                nc.vector.tensor_copy(vcp[:], vps[tb][:])
                vcps.append(vcp)

            ps1.__exit__(None, None, None)
            ps2 = tc.tile_pool(name="ps2", bufs=1, space="PSUM")
            PS["p"] = ps2.__enter__()
            # ---- pipelined norm+rope stages: Q first (their post_a ran
            # during phase 2, so PE enters the window with zero stall), K last
            qorder = [blocks[d] for dd in range(4) for d in (dd, dd + 4)]
            blocks = qorder + blocks[8:]
            for d in blocks:
                stage_s(d)
            # V transposes keep PE busy while ACT does the sqrts
            tps = []
            for tb in range(4):
                for c in range(4):
                    tp = PS["p"].tile([128, 128], f32, tag="tp", bufs=2,
                                      name="tp")
                    nc.tensor.transpose(
                        tp[:], vcps[tb][:, 128 * c:128 * (c + 1)], ident[:])
                    tps.append(tp)
            for d in blocks:
                stage_sqrt(d)
            for i, tp in enumerate(tps):
                nc.vector.tensor_copy(vpr[0][:, i, 0:64], tp[:, 0:64])
                nc.vector.tensor_copy(vpr[1][:, i, 0:64], tp[:, 64:128])
            for i, d in enumerate(blocks):
                stage_b(d)
                if i >= 2:
                    stage_qn(blocks[i - 2])
                    stage_rot(blocks[i - 2])
                    stage_rope(blocks[i - 2])
            for d in blocks[-2:]:
                stage_qn(d)
                stage_rot(d)
                stage_rope(d)

            for p in range(4):
                nc.sync.dma_start(wot[p][:], wot_d[128 * p:128 * (p + 1), :])
            ps2.__exit__(None, None, None)
            ps3 = tc.tile_pool(name="ps3", bufs=1, space="PSUM")
            PS["p"] = ps3.__enter__()

            # ================= Phase 3: attention + Wo =================
            def emit_norm(hd):
                avp, pair, off, ssl = hd
                cs = wk_pool.tile([1, 512], f32r, tag="cs", bufs=2, name="cs")
                nc.vector.tensor_copy(cs[:], avp[64:65, :])
                bcp = PS["p"].tile([64, 512], f32, tag="bcp", bufs=1,
                                   name="bcp")
                nc.tensor.matmul(bcp[:], bones[:], cs[:], start=True,
                                 stop=True)
                rcb = wk_pool.tile([64, 512], f32, tag="rcb", bufs=2,
                                   name="rcb")
                nc.vector.reciprocal_approx_fast(rcb[:], bcp[:])
                nc.vector.tensor_mul(attn[pair][off:off + 64, ssl],
                                     avp[0:64, :], rcb[:])

            def wo_st(s0):
                osb = wk_pool.tile([128, HID], f32, tag="osb", bufs=2,
                                   name="osb")
                for eb in range(4):
                    wops = PS["p"].tile([128, 512], f32, tag="scp",
                                        bufs=3, name="wops")
                    for p in range(4):
                        nc.tensor.matmul(
                            wops[:],
                            attn[p][:, s0:s0 + 128],
                            wot[p][:, 512 * eb:512 * (eb + 1)],
                            start=(p == 0), stop=(p == 3))
                    nc.vector.tensor_copy(
                        osb[:, 512 * eb:512 * (eb + 1)], wops[:])
                nc.sync.dma_start(o_d[s0:s0 + 128, :], osb[:])

            def attention(p4, ssl):
                prev_heads = []
                if True:
                    # head pair (p4, p4+4): kv groups 0/1 at partition bases
                    # 0/64 -> QK pairs run concurrently in separate PE
                    # row-groups
                    qsA = qf[p4][0:64, ssl]
                    qsB = qf[p4][64:128, ssl]
                    avpA = PS["p"].tile([65, 512], f32, tag="avp", bufs=4,
                                        name="avpA")
                    avpB = PS["p"].tile([65, 512], f32, tag="avp", bufs=4,
                                        name="avpB")
                    expsA, expsB = [], []
                    for tcn in range(16):
                        tsl = slice(128 * tcn, 128 * (tcn + 1))
                        scpA = PS["p"].tile([128, 512], f32, tag="scp",
                                            bufs=3, name="scpA")
                        nc.tensor.matmul(scpA[:], kf[0:64, tsl], qsA,
                                         start=True, stop=True)
                        scpB = PS["p"].tile([128, 512], f32, tag="scp",
                                            bufs=3, name="scpB")
                        nc.tensor.matmul(scpB[:], kf[64:128, tsl], qsB,
                                         start=True, stop=True)
                        eA = wk_pool.tile([128, 512], bf16, tag="exp", bufs=8,
                                          name="expA")
                        nc.scalar.activation(eA[:], scpA[:], Act.Exp,
                                             scale=0.125)
                        eB = wk_pool.tile([128, 512], bf16, tag="exp", bufs=8,
                                          name="expB")
                        nc.scalar.activation(eB[:], scpB[:], Act.Exp,
                                             scale=0.125)
                        expsA.append(eA)
                        expsB.append(eB)
                        if tcn == 3:
                            for hd in prev_heads:
                                emit_norm(hd)
                            prev_heads = []
                        if tcn >= 1:
                            nc.tensor.matmul(avpA[:], vpr[0][:, tcn - 1, 0:65],
                                             expsA[tcn - 1][:],
                                             start=(tcn == 1), stop=False)
                            nc.tensor.matmul(avpB[:], vpr[1][:, tcn - 1, 0:65],
                                             expsB[tcn - 1][:],
                                             start=(tcn == 1), stop=False)
                    nc.tensor.matmul(avpA[:], vpr[0][:, 15, 0:65],
                                     expsA[15][:], start=False, stop=True)
                    nc.tensor.matmul(avpB[:], vpr[1][:, 15, 0:65],
                                     expsB[15][:], start=False, stop=True)
                    prev_heads = [(avpA, p4, 0, ssl), (avpB, p4, 64, ssl)]
                for hd in prev_heads:
                    emit_norm(hd)
                prev_heads = []

            ssl0 = slice(0, 512)
            ssl1 = slice(512, 1024)
            for p4 in range(4):
                attention(p4, ssl0)
            for p4 in range(4):
                attention(p4, ssl1)
                # Wo for s-block 0 rides under sb1's ACT-bound stream
                wo_st(128 * p4)
            for st in range(4):
                wo_st(512 + 128 * st)
            if DEBUG:
                for d in range(4):
                    nc.sync.dma_start(qf_dbg[d], qf[d][:])
                    nc.sync.dma_start(attn_dbg[d], attn[d][:])
                nc.sync.dma_start(kf_dbg[:], kf[:])
                for g in range(2):
                    nc.sync.dma_start(vpr_dbg[g], vpr[g][:, :, 0:65])
            ps3.__exit__(None, None, None)

    nc.compile()
    return nc


def _host_inputs(hidden_states, target_context, cos, sin, Wq, Wk, Wv, Wo,
                 q_gamma, k_gamma):
    """Build the 8 per-core input maps."""
    f32 = np.float32
    P = np.zeros((128, 128), dtype=f32)
    for d in range(128):
        base, dd = (d // 64) * 64, d % 64
        if dd < 32:
            P[d, base + dd + 32] = -1.0
        else:
            P[d, base + dd - 32] = 1.0
    p_lhsT = np.ascontiguousarray(P.T)
    onesblk = np.zeros((128, 2), dtype=f32)
    onesblk[0:64, 0] = 1.0
    onesblk[64:128, 1] = 1.0
    ident = np.eye(128, dtype=f32)
    bones = np.ones((1, 64), dtype=f32)

    qg_rot = np.roll(q_gamma, -32)
    kg_rot = np.roll(k_gamma, -32)
    # head permutation: tile d holds heads (d, d+4) so that each head's
    # partition offset matches its kv-group offset in kf
    perm = np.concatenate(
        [np.arange(64 * h, 64 * h + 64) for h in (0, 4, 1, 5, 2, 6, 3, 7)])

    in_maps = []
    for core in range(8):
        b, hg = core // 4, core % 4
        if core % 4 == 0:
            xt = np.ascontiguousarray(
                np.concatenate([target_context[b], hidden_states[b]], 0).T
            ).astype(f32, copy=False)
            cosq = np.ascontiguousarray(
                np.tile((cos[b, CTX:] * q_gamma).T, (2, 1))).astype(f32)
            sinq = np.ascontiguousarray(
                np.tile((sin[b, CTX:] * qg_rot).T, (2, 1))).astype(f32)
            cosk = np.ascontiguousarray(
                np.tile((cos[b, :T] * k_gamma).T, (2, 1))).astype(f32)
            sink = np.ascontiguousarray(
                np.tile((sin[b, :T] * kg_rot).T, (2, 1))).astype(f32)
        wqt = np.ascontiguousarray(
            Wq[512 * hg:512 * (hg + 1), :][perm, :].T).astype(f32)
        wkt = np.ascontiguousarray(Wk[128 * hg:128 * (hg + 1), :].T).astype(f32)
        wvt = np.ascontiguousarray(Wv[128 * hg:128 * (hg + 1), :].T).astype(f32)
        wot = np.ascontiguousarray(
            Wo[:, 512 * hg:512 * (hg + 1)].T[perm, :]).astype(
                ml_dtypes.bfloat16)
        in_maps.append({
            "xt": xt, "wqt": wqt, "wkt": wkt, "wvt": wvt, "wot": wot,
            "cosq": cosq, "sinq": sinq, "cosk": cosk, "sink": sink,
            "prot": p_lhsT, "onesblk": onesblk, "ident": ident, "bones": bones,
        })
    return in_maps


def _run(in_maps, trace=False, trace_kwargs=None):
    from concourse.bass_utils import run_bass_kernel_spmd
    if "nc" not in _CACHE:
        _CACHE["nc"] = _build()
    kw = {}
    if trace:
        kw["trace"] = True
        if trace_kwargs:
            kw["trace_kwargs"] = trace_kwargs
    return run_bass_kernel_spmd(_CACHE["nc"], in_maps, list(range(8)), **kw)


def kernel(hidden_states, target_context, cos, sin, Wq, Wk, Wv, Wo,
           q_gamma, k_gamma, _trace=False):
    in_maps = _host_inputs(
        np.asarray(hidden_states, np.float32),
        np.asarray(target_context, np.float32),
        np.asarray(cos, np.float32), np.asarray(sin, np.float32),
        np.asarray(Wq, np.float32), np.asarray(Wk, np.float32),
        np.asarray(Wv, np.float32), np.asarray(Wo, np.float32),
        np.asarray(q_gamma, np.float32), np.asarray(k_gamma, np.float32))
    res = _run(in_maps, trace=_trace)
    out = np.zeros((B, S, HID), dtype=np.float32)
    for core in range(8):
        out[core // 4] += res.results[core]["o"]
    if _trace:
        return out, res
    return out
